# revision 19
# baseline (speedup 1.0000x reference)
"""Trainium2 Bass kernel for the 4-layer sum/product circuit
(nn_KnowledgeLayer): h = enc(x); h = h[idx0].prod(1); h = h[idx1].sum(1);
h = h[idx2].prod(1); h = h[idx3].sum(1).

Strategy (shard the COMPOSED SLOT STREAM, not the batch):
  * Host composes the four index maps into TWO flat operand streams of
    32768 row-indices each into a 4098-row full-batch enc table
    ([x | 1-x | 0 | 1], built host-side as [4098, 1024] fp16).
  * Core c owns h3 rows [c*512, (c+1)*512) and gathers FULL 2KB rows
    at HBM line rate: 8 chunks x 64 outputs, 2 gather calls per chunk
    on rotating SWDGE queues, 6 gather buffers (3 chunks in flight).
  * DMA-instruction budget is kept inside the Tile DMA-semaphore pool
    (1 meta load + 16 gathers on 4 shared queue sems + 8 output
    stores), and the meta tile is immediately copied by DVE so the
    load semaphore's only waiter retires early -- no recycle barrier
    can bind the tail to the gather stream.
  * Slot position g = j*128 + p, j = cb*2 + b, p = ii*2 + a; the DVE
    tree reduction is fully in-place in the h0 tile (cb-sum into
    blocks 0:2, b-prod into block 0), the final a-sum pairs adjacent
    PARTITIONS via a PE matmul with a [128, 64] pairing matrix (f32
    PSUM); ACT drains PSUM to fp16; one fp16 output DMA per chunk
    (host upcasts to f32).

The bass program is identical for all 8 cores (pure SPMD); per-core
index streams differ via in_maps.
"""

import numpy as np

N_VARS = 2048
BATCH = 1024
NCORES = 8
TABLE = 2 * N_VARS + 2            # 4098
NOUT = 4096                       # h3 rows total
CORE_OUT = NOUT // NCORES         # 512 h3 rows per core
NCHUNK = 8
CHO = CORE_OUT // NCHUNK          # 64 h3 rows per chunk
CHS = CHO * 8                     # 512 h0 slots per chunk per stream
ICOLS = CORE_OUT * 8 // 16        # 256 idx columns per stream


# ----------------------------------------------------------------------------
# host-side index preparation
# ----------------------------------------------------------------------------

def _remap(e):
    """reference enc row -> our table row.
    table: [0,2048) = x[f], [2048,4096) = 1-x[f], 4096 = 0, 4097 = 1."""
    out = np.empty_like(e)
    out[e == 0] = 2 * N_VARS
    out[e == 1] = 2 * N_VARS + 1
    even = (e >= 2) & (e % 2 == 0)
    out[even] = (e[even] - 2) // 2
    odd = (e >= 3) & (e % 2 == 1)
    out[odd] = N_VARS + (e[odd] - 3) // 2
    return out


def _compose_indices(idx0, idx1, idx2, idx3):
    J = idx3.reshape(-1)              # [8192]  (i, a)   layer3 sum pairs
    K = idx2[J].reshape(-1)           # [16384] (i, a, b) layer2 prod pairs
    L = idx1[K].reshape(-1)           # [32768] (i, a, b, c) layer1 sum pairs
    AB = idx0[L]                      # [32768, 2]       layer0 prod pairs
    A = _remap(AB[:, 0].astype(np.int64))
    B = _remap(AB[:, 1].astype(np.int64))
    return A.reshape(NOUT, 2, 2, 2), B.reshape(NOUT, 2, 2, 2)


def _core_wrap(S, c):
    """Per-core chunked+wrapped int16 index tensor [128, ICOLS].

    Chunk k covers i = c*512 + k*64 + ii.  Gather position within a call:
    g = j*128 + p with free block j = cbit*2 + b and partition p = ii*2+a,
    so h1 = h0[:, :2]+h0[:, 2:], h2 = h1[:, :1]*h1[:, 1:2], and the final
    a-sum pairs adjacent partitions (PE matmul).
    SWDGE wraps each call's g-stream: idx[p16, s] = call[s*16 + p16].
    """
    Sc = S[c * CORE_OUT:(c + 1) * CORE_OUT]              # [512, 2, 2, 2]
    Sc = Sc.reshape(NCHUNK, CHO, 2, 2, 2)                # [k, ii, a, b, cb]
    Sc = Sc.transpose(0, 4, 3, 1, 2)                     # [k, cb, b, ii, a]
    calls = Sc.reshape(NCHUNK, CHS)                      # g = ((cb*2+b)*64+ii)*2+a
    w = calls.reshape(NCHUNK, CHS // 16, 16)             # [k, s, p16]
    w = w.transpose(2, 0, 1).astype(np.int16)            # [16, k, s]
    w = w.reshape(16, ICOLS)
    return np.ascontiguousarray(np.tile(w, (8, 1)))      # [128, ICOLS]


# ----------------------------------------------------------------------------
# bass program (built once, cached)
# ----------------------------------------------------------------------------

_CACHED = {}


def _build_program():
    import concourse.bacc as bacc
    import concourse.mybir as mybir
    from concourse.tile import TileContext

    f32 = mybir.dt.float32
    f16 = mybir.dt.float16
    i16 = mybir.dt.int16

    nc = bacc.Bacc("TRN2", target_bir_lowering=False, debug=False,
                   num_swdge_queues=4)

    enc = nc.dram_tensor("enc", [TABLE, BATCH], f16, kind="ExternalInput")
    # meta packs idxa [128,256] i16, idxb [128,256] i16, pairs [128,64] f16
    meta = nc.dram_tensor("meta", [128, 2 * ICOLS + 64], i16,
                          kind="ExternalInput")
    # out[ii, k, :] = h3 row k*64+ii of this core (host reorders)
    out = nc.dram_tensor("out", [CHO, NCHUNK, BATCH], f16,
                         kind="ExternalOutput")

    with TileContext(nc) as tc:
        with tc.tile_pool(name="setup", bufs=1) as sp, \
             tc.tile_pool(name="gather", bufs=8) as gp, \
             tc.tile_pool(name="mid", bufs=2) as mp, \
             tc.tile_pool(name="hpsum", bufs=3, space="PSUM") as pp:

            mt = sp.tile([128, 2 * ICOLS + 64], i16, tag="mt")
            nc.sync.dma_start(out=mt[:, :], in_=meta[:, :])
            # copy meta so the load-DMA sem's only waiter retires early;
            # all gathers depend on the copy (a DVE counter), not the DMA sem
            mt2 = sp.tile([128, 2 * ICOLS + 64], i16, tag="mt2")
            nc.vector.tensor_copy(mt2[:, :], mt[:, :])
            ia = mt2[:, 0:ICOLS]
            ib = mt2[:, ICOLS:2 * ICOLS]
            pr = mt2[:, 2 * ICOLS:].bitcast(f16)
            cnt = nc.gpsimd.to_reg(CHS)
            cnt2 = nc.gpsimd.to_reg(CHS // 2)
            otf = sp.tile([CHO, NCHUNK, BATCH], f16, tag="otf")

            ccols = CHS // 16        # 32 idx columns per chunk
            for k in range(NCHUNK):
                h0 = mp.tile([128, 4, BATCH], f16, tag="h0")
                h1 = mp.tile([128, 2, BATCH], f16, tag="h1")
                h2 = mp.tile([128, 1, BATCH], f16, tag="h2")
                ps = pp.tile([CHO, BATCH], f32, tag="ps")
                isl = ia[:, k * ccols:(k + 1) * ccols]
                jsl = ib[:, k * ccols:(k + 1) * ccols]
                if k == 0:
                    # first chunk: gather by COLUMN halves on all 4 queues so
                    # DVE can start on columns 0:512 while 512:1024 transfer
                    for half in range(2):
                        cs = slice(half * 512, (half + 1) * 512)
                        gah = gp.tile([128, 4, 512], f16, tag="ga",
                                      name="gah")
                        gbh = gp.tile([128, 4, 512], f16, tag="gb",
                                      name="gbh")
                        nc.gpsimd.dma_gather(
                            out_ap=gah[:, :, :], in_ap=enc[:, cs],
                            idxs_ap=isl, num_idxs=CHS, num_idxs_reg=cnt,
                            elem_size=512, elem_step=BATCH,
                            queue_num=2 * half)
                        nc.gpsimd.dma_gather(
                            out_ap=gbh[:, :, :], in_ap=enc[:, cs],
                            idxs_ap=jsl, num_idxs=CHS, num_idxs_reg=cnt,
                            elem_size=512, elem_step=BATCH,
                            queue_num=2 * half + 1)
                        nc.vector.tensor_mul(h0[:, :, cs], gah[:, :, :],
                                             gbh[:, :, :])
                        nc.vector.tensor_add(
                            h1[:, :, cs], h0[:, 0:2, cs], h0[:, 2:4, cs])
                        nc.vector.tensor_mul(
                            h2[:, :, cs], h1[:, 0:1, cs], h1[:, 1:2, cs])
                        nc.tensor.matmul(
                            ps[:, cs], lhsT=pr, rhs=h2[:, 0, cs],
                            start=True, stop=True)
                else:
                    ga = gp.tile([128, 4, BATCH], f16, tag="ga")
                    gb = gp.tile([128, 4, BATCH], f16, tag="gb")
                    # 4 calls per chunk (idx-halves) keep several transfers
                    # in flight so the stream stays at fabric rate
                    hc = ccols // 2
                    for st, gt, q0 in ((isl, ga, 2 * k), (jsl, gb, 2 * k + 1)):
                        nc.gpsimd.dma_gather(
                            out_ap=gt[:, 0:2, :], in_ap=enc[:, :],
                            idxs_ap=st[:, 0:hc],
                            num_idxs=CHS // 2, num_idxs_reg=cnt2,
                            elem_size=BATCH, queue_num=q0 % 4)
                        nc.gpsimd.dma_gather(
                            out_ap=gt[:, 2:4, :], in_ap=enc[:, :],
                            idxs_ap=st[:, hc:ccols],
                            num_idxs=CHS // 2, num_idxs_reg=cnt2,
                            elem_size=BATCH, queue_num=(q0 + 2) % 4)
                    nc.vector.tensor_mul(h0[:, :, :], ga[:, :, :],
                                         gb[:, :, :])
                    nc.vector.tensor_add(
                        h1[:, :, :], h0[:, 0:2, :], h0[:, 2:4, :])
                    nc.vector.tensor_mul(
                        h2[:, :, :], h1[:, 0:1, :], h1[:, 1:2, :])
                    for half in range(2):
                        cs = slice(half * 512, (half + 1) * 512)
                        nc.tensor.matmul(
                            ps[:, cs], lhsT=pr, rhs=h2[:, 0, cs],
                            start=True, stop=True)
                nc.scalar.copy(otf[:, k, :], ps[:, :])
                if k in (NCHUNK // 2 - 1, NCHUNK - 1):
                    lo = 0 if k < NCHUNK // 2 else NCHUNK // 2
                    nc.sync.dma_start(
                        out=out[:, lo:k + 1, :], in_=otf[:, lo:k + 1, :])

    nc.compile()
    return nc


def _get_program():
    if "nc" not in _CACHED:
        _CACHED["nc"] = _build_program()
    return _CACHED["nc"]


# ----------------------------------------------------------------------------
# public entry point
# ----------------------------------------------------------------------------

def kernel(x, idx0, idx1, idx2, idx3, _trace=False, _trace_kwargs=None):
    from concourse.bass_utils import run_bass_kernel_spmd

    x = np.ascontiguousarray(np.asarray(x, dtype=np.float32))
    A, B = _compose_indices(
        np.asarray(idx0), np.asarray(idx1), np.asarray(idx2), np.asarray(idx3))

    enc = np.concatenate(
        [x, 1.0 - x,
         np.zeros((1, BATCH), np.float32),
         np.ones((1, BATCH), np.float32)], axis=0)
    enc = np.ascontiguousarray(enc.astype(np.float16))

    pairs = np.zeros((128, 64), np.float16)
    pairs[np.arange(128), np.arange(128) // 2] = 1.0

    nc = _get_program()
    in_maps = []
    for c in range(NCORES):
        mt = np.concatenate(
            [_core_wrap(A, c), _core_wrap(B, c), pairs.view(np.int16)], axis=1)
        in_maps.append({"enc": enc, "meta": np.ascontiguousarray(mt)})

    kwargs = {}
    if _trace:
        kwargs["trace"] = True
        if _trace_kwargs:
            kwargs.update(_trace_kwargs)
    res = run_bass_kernel_spmd(nc, in_maps, core_ids=list(range(NCORES)), **kwargs)
    outs = [res.results[c]["out"].transpose(1, 0, 2).reshape(CORE_OUT, BATCH)
            .astype(np.float32) for c in range(NCORES)]
    full = np.concatenate(outs, axis=0)
    if _trace:
        kernel.last_exec_time_ns = res.exec_time_ns
        kernel.last_profile = res.profile_json
    return full
